# revision 16
# baseline (speedup 1.0000x reference)
"""AttentionPooling (segment softmax-pool) Trainium2 kernel — v2 (bf16 batched).

out[s,:] = sum_n 1[idx[n]==s] * gnorm[n] * (x[n,:] @ msg_w + msg_b)
  gnorm[n] = w[n]^p * exp(gate[n]) / (denom[seg] + eps)   (max-sub skipped:
  mathematically identical after normalization, logits are O(5))

Restructured so the big matmul contracts rows via a one-hot:
  A[s,d]   = sum_n G[n,s] * x[n,d],  denom[s] = sum_n G[n,s]   (ones col)
  out[s,:] = (A[s,:] @ msg_w) / (denom+eps) + (denom/(denom+eps)) * msg_b
where G[n,s] = 1[idx[n]==s] * g[n].

v2 changes vs v1 (647us):
- Everything bf16 on the wire and in the hot loop: x tiles, gw, mask, G.
  fp32 matmul streams at half rate on the PE; bf16 halves both DMA bytes
  and PE time, and unlocks DVE 2x_1p mode for tensor_tensor.
- Logit chain batched per GROUP=32 tiles: one TT mult (2x) + two tree adds
  (2x) + one small reduce, instead of per-group big mult + 1x reduce.
- G-build batched: one ACT broadcast-exp (stride-0 input AP) materializes
  exp(logit2) across the WIN columns, then a single DVE TT mult against the
  u8->bf16 DMA-cast mask builds all GROUP G tiles in one op. Replaces 816
  per-tile ACT copies (346ns each) per core.
- Variable tiles-per-window from the actual (sorted) index: TPW_w =
  max over cores of ceil(rows/128), so the SPMD program is identical on
  all cores but ~7% of padding work disappears.
- Phase-2 PSUM->SBUF copies moved to ACT (ScalarE is PSUM-adjacent).
- Tile layout padded to 130 cols so every per-tile bf16 block is 4B-aligned
  (260B), keeping DVE 2x mode eligible on the sliced 3D access patterns.
"""

import os
import sys
import numpy as np

for _p in ("/opt/trn_rl_repo", "/root/.axon_site/_ro/trn_rl_repo"):
    if os.path.isdir(_p) and _p not in sys.path:
        sys.path.insert(0, _p)

import ml_dtypes

BF16 = ml_dtypes.bfloat16

P = 128
S = 16384
D = 128
TD = D + 2                     # tile width: 128 feats + ones col + pad col
NCORES = 8
WIN = 64                       # segments per PSUM window
NWIN = S // WIN                # 256 global windows
NWIN_CORE = NWIN // NCORES     # 32 per core
GROUP = 48                     # tiles per DMA/logit batch
EPS = 1e-10

U8CAST = True                  # ship mask u8, SWDGE dma casts to bf16 on load
ACT_RED = ()                   # groups whose reduce runs on ACT (accum) — the
                               # hidden ACTIVATION_READ_ACCUMULATOR slice makes
                               # this a net loss (measured 445ns/tile)
GPS_MASK = ()                  # groups whose mask-mult runs on GPSIMD — big
                               # gpsimd streaming ops slow DVE ~15% via the
                               # shared SBUF port (measured v4); keep empty

LAST_EXEC_NS = None
LAST_RESULTS = None

_module_cache = {}


def _build_module(tpw):
    """tpw: tuple of NWIN_CORE tile counts (same for all cores)."""
    key = ("v7", GROUP, ACT_RED, GPS_MASK, tuple(tpw))
    if key in _module_cache:
        return _module_cache[key]

    import concourse.bass as bass  # noqa: F401
    import concourse.tile as tile
    from concourse import bacc, mybir
    from concourse.masks import make_identity

    f32 = mybir.dt.float32
    bf16 = mybir.dt.bfloat16
    AX = mybir.AxisListType
    ALU = mybir.AluOpType
    ACTF = mybir.ActivationFunctionType

    NT = int(sum(tpw))
    NG = (NT + GROUP - 1) // GROUP
    NTP = NG * GROUP

    # flat tile stream: (window, first, last) per real tile
    tiles = []
    for w, c in enumerate(tpw):
        for k in range(c):
            tiles.append((w, k == 0, k == c - 1))

    nc = bacc.Bacc(
        "TRN2",
        target_bir_lowering=False,
        debug=False,
        enable_asserts=True,
        num_devices=NCORES,
    )

    xp = nc.dram_tensor("xp", [NG * P, GROUP * TD], bf16, kind="ExternalInput")
    mdt = mybir.dt.uint8 if U8CAST else bf16
    maskg = nc.dram_tensor("maskg", [NG * P, GROUP * WIN], mdt, kind="ExternalInput")
    wall = nc.dram_tensor("wall", [P, NTP], f32, kind="ExternalInput")
    gwrep = nc.dram_tensor("gwrep", [P, GROUP * D], bf16, kind="ExternalInput")
    msgw = nc.dram_tensor("msgw", [D, D], f32, kind="ExternalInput")
    msgbrep = nc.dram_tensor("msgbrep", [P, D], f32, kind="ExternalInput")
    gatebrep = nc.dram_tensor("gatebrep", [P, 1], f32, kind="ExternalInput")
    prep = nc.dram_tensor("prep", [P, 1], f32, kind="ExternalInput")
    out = nc.dram_tensor("out", [NWIN_CORE * WIN, D], f32, kind="ExternalOutput")

    with tile.TileContext(nc) as tc:
        from contextlib import ExitStack

        with ExitStack() as ctx:
            const_pool = ctx.enter_context(tc.tile_pool(name="const", bufs=1))
            xs_pool = ctx.enter_context(tc.tile_pool(name="xs", bufs=4))
            mk_pool = ctx.enter_context(tc.tile_pool(name="mk", bufs=4))
            xw_pool = ctx.enter_context(tc.tile_pool(name="xw", bufs=2))
            tr_pool = ctx.enter_context(tc.tile_pool(name="tr", bufs=2))
            lg_pool = ctx.enter_context(tc.tile_pool(name="lg", bufs=6))
            g_pool = ctx.enter_context(tc.tile_pool(name="gm", bufs=3))
            ge_pool = ctx.enter_context(tc.tile_pool(name="ge", bufs=4))
            psA_pool = ctx.enter_context(tc.tile_pool(name="psA", bufs=4, space="PSUM"))
            ps2_pool = ctx.enter_context(tc.tile_pool(name="ps2", bufs=2, space="PSUM"))
            ph2_pool = ctx.enter_context(tc.tile_pool(name="ph2", bufs=3))

            gw_t = const_pool.tile([P, GROUP * D], bf16)
            nc.sync.dma_start(gw_t[:], gwrep[:, :])
            msgw_t = const_pool.tile([D, D], f32)
            nc.sync.dma_start(msgw_t[:], msgw[:, :])
            msgb_t = const_pool.tile([P, D], f32)
            nc.sync.dma_start(msgb_t[:], msgbrep[:, :])
            gateb_t = const_pool.tile([P, 1], f32)
            nc.sync.dma_start(gateb_t[:], gatebrep[:, :])
            p_t = const_pool.tile([P, 1], f32)
            nc.sync.dma_start(p_t[:], prep[:, :])
            ident = const_pool.tile([P, P], f32)
            make_identity(nc, ident[:])

            # hoisted: p*ln(w) for every tile in two ops
            w_t = const_pool.tile([P, NTP], f32)
            nc.sync.dma_start(w_t[:], wall[:, :])
            plw_t = const_pool.tile([P, NTP], f32)
            nc.scalar.activation(out=plw_t[:], in_=w_t[:], func=ACTF.Ln)
            nc.vector.tensor_scalar_mul(plw_t[:], plw_t[:], p_t[:, 0:1])

            gw3 = gw_t[:].rearrange("p (t d) -> p t d", d=D)

            chains = {}

            def emit_chain(g):
                gc = min(GROUP, NT - g * GROUP)
                xs = xs_pool.tile([P, gc * TD], bf16, tag="xs", name=f"xs{g}")
                nc.sync.dma_start(xs[:], xp[g * P : (g + 1) * P, 0 : gc * TD])
                xs3 = xs[:].rearrange("p (t d) -> p t d", d=TD)
                mk = mk_pool.tile([P, gc * WIN], bf16, tag="mk", name=f"mk{g}")
                if U8CAST:
                    nc.gpsimd.dma_start(mk[:], maskg[g * P : (g + 1) * P, 0 : gc * WIN])
                else:
                    nc.sync.dma_start(mk[:], maskg[g * P : (g + 1) * P, 0 : gc * WIN])
                xw = xw_pool.tile([P, gc * D], bf16, tag="xw", name=f"xw{g}")
                xw3 = xw[:].rearrange("p (t d) -> p t d", d=D)
                nc.vector.tensor_tensor(
                    out=xw3, in0=xs3[:, :, 0:D], in1=gw3[:, 0:gc, :], op=ALU.mult
                )
                logit = lg_pool.tile([P, gc], f32, tag="lg", name=f"lg{g}")
                t1 = tr_pool.tile([P, gc * 64], bf16, tag="t1", name=f"t1{g}")
                t13 = t1[:].rearrange("p (t d) -> p t d", d=64)
                nc.vector.tensor_tensor(
                    out=t13, in0=xw3[:, :, 0:64], in1=xw3[:, :, 64:128],
                    op=ALU.add,
                )
                t2 = tr_pool.tile([P, gc * 32], bf16, tag="t2", name=f"t2{g}")
                t23 = t2[:].rearrange("p (t d) -> p t d", d=32)
                nc.vector.tensor_tensor(
                    out=t23, in0=t13[:, :, 0:32], in1=t13[:, :, 32:64],
                    op=ALU.add,
                )
                t3 = tr_pool.tile([P, gc * 16], bf16, tag="t3", name=f"t3{g}")
                t33 = t3[:].rearrange("p (t d) -> p t d", d=16)
                nc.vector.tensor_tensor(
                    out=t33, in0=t23[:, :, 0:16], in1=t23[:, :, 16:32],
                    op=ALU.add,
                )
                nc.vector.reduce_sum(out=logit[:], in_=t33, axis=AX.X)
                logit2 = lg_pool.tile([P, gc], f32, tag="lg2", name=f"l2{g}")
                nc.vector.tensor_add(
                    logit2[:], logit[:], plw_t[:, g * GROUP : g * GROUP + gc]
                )
                gexb = ge_pool.tile([P, gc * WIN], bf16, tag="gexb", name=f"ge{g}")
                gexb3 = gexb[:].rearrange("p (t s) -> p t s", s=WIN)
                lg2b = logit2[:].unsqueeze(2).broadcast_to([P, gc, WIN])
                nc.scalar.activation(
                    out=gexb3, in_=lg2b, func=ACTF.Exp, bias=gateb_t[:, 0:1]
                )
                chains[g] = (xs3, mk, gexb, gc)

            def emit_gate(g):
                # deferred two groups behind emit_chain: by the time DVE reaches
                # this op in its in-order stream, ACT's exp(g) has had ample
                # time to land, so DVE doesn't stall on it
                xs3, mk, gexb, gc = chains[g]
                Gm = g_pool.tile([P, gc * WIN], bf16, tag="G", name=f"G{g}")
                eng = nc.gpsimd if g in GPS_MASK else nc.vector
                eng.tensor_tensor(out=Gm[:], in0=mk[:], in1=gexb[:], op=ALU.mult)
                chains[g] = (xs3, Gm)

            def emit_phase2(w, psA):
                sbA = ph2_pool.tile([WIN, D + 1], f32, tag="sbA", name=f"sbA{w}")
                nc.scalar.copy(sbA[:], psA[:])
                deno = ph2_pool.tile([WIN, 1], f32, tag="deno", name=f"dn{w}")
                nc.vector.tensor_scalar_add(deno[:], sbA[:, D : D + 1], EPS)
                rcp = ph2_pool.tile([WIN, 1], f32, tag="rcp", name=f"rc{w}")
                nc.vector.reciprocal(out=rcp[:], in_=deno[:])
                coef = ph2_pool.tile([WIN, 1], f32, tag="coef", name=f"cf{w}")
                nc.vector.tensor_tensor(
                    out=coef[:], in0=sbA[:, D : D + 1], in1=rcp[:], op=ALU.mult
                )
                psAT = ps2_pool.tile([P, WIN], f32, tag="AT", name=f"AT{w}")
                nc.tensor.transpose(
                    out=psAT[:], in_=sbA[:, 0:D], identity=ident[:WIN, :WIN]
                )
                sbAT = ph2_pool.tile([P, WIN], f32, tag="sbAT", name=f"sT{w}")
                nc.scalar.copy(sbAT[:], psAT[:])
                ps2 = ps2_pool.tile([WIN, D], f32, tag="out2", name=f"o2{w}")
                nc.tensor.matmul(
                    out=ps2[:], lhsT=sbAT[:], rhs=msgw_t[:], start=True, stop=True
                )
                outsb = ph2_pool.tile([WIN, D], f32, tag="outsb", name=f"ou{w}")
                nc.scalar.activation(
                    out=outsb[:], in_=ps2[:], func=ACTF.Copy, scale=rcp[:, 0:1]
                )
                bterm = ph2_pool.tile([WIN, D], f32, tag="bterm", name=f"bt{w}")
                nc.scalar.activation(
                    out=bterm[:], in_=msgb_t[:WIN, :], func=ACTF.Copy,
                    scale=coef[:, 0:1],
                )
                ofin = ph2_pool.tile([WIN, D], f32, tag="ofin", name=f"of{w}")
                nc.vector.tensor_add(ofin[:], outsb[:], bterm[:])
                nc.sync.dma_start(out[w * WIN : (w + 1) * WIN, :], ofin[:])

            cur = {}
            emit_chain(0)
            if NG > 1:
                emit_chain(1)
            for g in range(NG):
                if g + 2 < NG:
                    emit_chain(g + 2)
                emit_gate(g)
                xs3, Gm = chains.pop(g)
                for j in range(GROUP):
                    t = g * GROUP + j
                    if t >= NT:
                        break
                    w, first, last = tiles[t]
                    if first:
                        cur[w] = psA_pool.tile(
                            [WIN, D + 1], f32, tag="psA", name=f"psA{w}"
                        )
                    nc.tensor.matmul(
                        out=cur[w][:],
                        lhsT=Gm[:, j * WIN : (j + 1) * WIN],
                        rhs=xs3[:, j, 0 : D + 1],
                        start=first,
                        stop=last,
                    )
                    if last:
                        emit_phase2(w, cur.pop(w))

    nc.compile()
    _module_cache[key] = (nc, NT, NG)
    return _module_cache[key]


def _layout(x, idx, w):
    """Pad + reorder host arrays into the per-core device layouts.

    Returns (tpw, xdev, maskdev, wdev)."""
    n = idx.shape[0]
    bounds = np.searchsorted(idx, np.arange(0, S + 1, WIN)).astype(np.int64)
    counts = np.diff(bounds)                       # rows per global window [NWIN]
    cpw = counts.reshape(NCORES, NWIN_CORE)
    tpw = np.maximum(1, -(-cpw // P)).max(axis=0)  # tiles per window, shared
    NT = int(tpw.sum())
    NG = (NT + GROUP - 1) // GROUP
    NTP = NG * GROUP
    ROWS_CORE = NTP * P

    tile_off = np.zeros(NWIN_CORE + 1, dtype=np.int64)
    np.cumsum(tpw, out=tile_off[1:])

    wg = np.repeat(np.arange(NWIN, dtype=np.int64), counts)     # global window
    rank = np.arange(n, dtype=np.int64) - np.repeat(bounds[:-1], counts)
    core = wg // NWIN_CORE
    wl = wg % NWIN_CORE
    dest = core * ROWS_CORE + tile_off[wl] * P + rank

    xpad = np.zeros((NCORES * ROWS_CORE, TD), dtype=np.float32)
    xpad[:, D] = 1.0
    xpad[dest, 0:D] = x
    segl = (idx - wg * WIN).astype(np.int64)

    mask = np.zeros((NCORES * ROWS_CORE, WIN), dtype=np.uint8)
    mask[dest, segl] = 1
    wpad = np.ones(NCORES * ROWS_CORE, dtype=np.float32)
    wpad[dest] = w

    xdev = (
        xpad.astype(BF16)
        .reshape(NCORES, NG, GROUP, P, TD)
        .transpose(0, 1, 3, 2, 4)
        .reshape(NCORES, NG * P, GROUP * TD)
    )
    maskdev = (
        mask.reshape(NCORES, NG, GROUP, P, WIN)
        .transpose(0, 1, 3, 2, 4)
        .reshape(NCORES, NG * P, GROUP * WIN)
    )
    if not U8CAST:
        maskdev = maskdev.astype(BF16)
    wdev = np.ascontiguousarray(
        wpad.reshape(NCORES, NTP, P).transpose(0, 2, 1)
    )
    return tuple(int(t) for t in tpw), xdev, maskdev, wdev


def _ensure_ntff_hook():
    """The image's antenv package lacks axon_hooks; shim it so trace=True
    can register the ctypes NTFF hook from trn_agent_boot."""
    try:
        from antenv.axon_hooks import get_axon_ntff_profile_hook  # noqa: F401

        return True
    except ImportError:
        pass
    try:
        import types

        import antenv
        from trn_agent_boot.trn_boot import _ntff_profile_via_ctypes

        mod = types.ModuleType("antenv.axon_hooks")
        _hook = [None]
        mod.set_axon_ntff_profile_hook = lambda h: _hook.__setitem__(0, h)
        mod.get_axon_ntff_profile_hook = lambda: _hook[0]
        sys.modules["antenv.axon_hooks"] = mod
        antenv.axon_hooks = mod
        mod.set_axon_ntff_profile_hook(
            _ntff_profile_via_ctypes("/opt/axon/libaxon_pjrt.so")
        )
        return True
    except Exception as e:  # degrade to untraced run
        print(f"ntff hook install failed: {type(e).__name__}: {e}")
        return False


def kernel(x, index, weights, gate_w, gate_b, msg_w, msg_b, pow_p):
    global LAST_EXEC_NS, LAST_RESULTS

    x = np.ascontiguousarray(np.asarray(x, dtype=np.float32))
    idx = np.asarray(index).astype(np.int64).ravel()
    w = np.asarray(weights, dtype=np.float32).ravel()
    gate_w = np.asarray(gate_w, dtype=np.float32).reshape(D)
    gate_b = np.asarray(gate_b, dtype=np.float32).reshape(1)
    msg_w = np.ascontiguousarray(np.asarray(msg_w, dtype=np.float32))
    msg_b = np.asarray(msg_b, dtype=np.float32).reshape(D)
    pow_p = np.asarray(pow_p, dtype=np.float32).reshape(1)

    if not np.all(idx[1:] >= idx[:-1]):
        perm = np.argsort(idx, kind="stable")
        idx = idx[perm]
        x = x[perm]
        w = w[perm]

    tpw, xdev, maskdev, wdev = _layout(x, idx, w)

    gwrep = np.tile(gate_w[None, :], (P, GROUP)).astype(BF16)
    msgbrep = np.tile(msg_b[None, :], (P, 1)).astype(np.float32)
    gatebrep = np.full((P, 1), gate_b[0], dtype=np.float32)
    prep = np.full((P, 1), pow_p[0], dtype=np.float32)
    nc, NT, NG = _build_module(tpw)
    from concourse.bass_utils import run_bass_kernel_spmd

    in_maps = []
    for c in range(NCORES):
        in_maps.append(
            {
                "xp": np.ascontiguousarray(xdev[c]),
                "maskg": np.ascontiguousarray(maskdev[c]),
                "wall": wdev[c],
                "gwrep": gwrep,
                "msgw": msg_w,
                "msgbrep": msgbrep,
                "gatebrep": gatebrep,
                "prep": prep,
            }
        )

    trace = bool(os.environ.get("KERNEL_TRACE"))
    if trace:
        trace = _ensure_ntff_hook()
    res = run_bass_kernel_spmd(
        nc, in_maps, core_ids=list(range(NCORES)), trace=trace
    )
    LAST_RESULTS = res
    LAST_EXEC_NS = res.exec_time_ns

    out = np.concatenate([res.results[c]["out"] for c in range(NCORES)], axis=0)
    return out.astype(np.float32)


def kernel_numpy(x, index, weights, gate_w, gate_b, msg_w, msg_b, pow_p):
    """Host-side mirror of the device algorithm (debug only)."""
    x = np.asarray(x, dtype=np.float32)
    idx = np.asarray(index).astype(np.int64).ravel()
    w = np.asarray(weights, dtype=np.float32).ravel()
    xb = x.astype(BF16).astype(np.float32)
    gwb = np.asarray(gate_w, dtype=np.float32).astype(BF16).astype(np.float32)
    xw = (xb * gwb.reshape(1, D)).astype(BF16).astype(np.float32)
    t1 = (xw[:, 0:64] + xw[:, 64:128]).astype(BF16).astype(np.float32)
    t2 = (t1[:, 0:32] + t1[:, 32:64]).astype(BF16).astype(np.float32)
    gate = t2.sum(axis=1) + np.asarray(gate_b).reshape(1)[0]
    g = np.exp(gate + np.asarray(pow_p).reshape(1)[0] * np.log(w))
    g = g.astype(BF16).astype(np.float32)
    A = np.zeros((S, D), dtype=np.float64)
    den = np.zeros(S, dtype=np.float64)
    np.add.at(A, idx, g[:, None] * xb)
    np.add.at(den, idx, g)
    out = (A @ np.asarray(msg_w, dtype=np.float64)) / (den[:, None] + EPS)
    out = out + (den / (den + EPS))[:, None] * np.asarray(msg_b).reshape(1, D)
    return out.astype(np.float32)


# revision 17
# speedup vs baseline: 1.1458x; 1.1458x over previous
"""AttentionPooling (segment softmax-pool) Trainium2 kernel — v2 (bf16 batched).

out[s,:] = sum_n 1[idx[n]==s] * gnorm[n] * (x[n,:] @ msg_w + msg_b)
  gnorm[n] = w[n]^p * exp(gate[n]) / (denom[seg] + eps)   (max-sub skipped:
  mathematically identical after normalization, logits are O(5))

Restructured so the big matmul contracts rows via a one-hot:
  A[s,d]   = sum_n G[n,s] * x[n,d],  denom[s] = sum_n G[n,s]   (ones col)
  out[s,:] = (A[s,:] @ msg_w) / (denom+eps) + (denom/(denom+eps)) * msg_b
where G[n,s] = 1[idx[n]==s] * g[n].

v2 changes vs v1 (647us):
- Everything bf16 on the wire and in the hot loop: x tiles, gw, mask, G.
  fp32 matmul streams at half rate on the PE; bf16 halves both DMA bytes
  and PE time, and unlocks DVE 2x_1p mode for tensor_tensor.
- Logit chain batched per GROUP=32 tiles: one TT mult (2x) + two tree adds
  (2x) + one small reduce, instead of per-group big mult + 1x reduce.
- G-build batched: one ACT broadcast-exp (stride-0 input AP) materializes
  exp(logit2) across the WIN columns, then a single DVE TT mult against the
  u8->bf16 DMA-cast mask builds all GROUP G tiles in one op. Replaces 816
  per-tile ACT copies (346ns each) per core.
- Variable tiles-per-window from the actual (sorted) index: TPW_w =
  max over cores of ceil(rows/128), so the SPMD program is identical on
  all cores but ~7% of padding work disappears.
- Phase-2 PSUM->SBUF copies moved to ACT (ScalarE is PSUM-adjacent).
- Tile layout padded to 130 cols so every per-tile bf16 block is 4B-aligned
  (260B), keeping DVE 2x mode eligible on the sliced 3D access patterns.
"""

import os
import sys
import numpy as np

for _p in ("/opt/trn_rl_repo", "/root/.axon_site/_ro/trn_rl_repo"):
    if os.path.isdir(_p) and _p not in sys.path:
        sys.path.insert(0, _p)

import ml_dtypes

BF16 = ml_dtypes.bfloat16

P = 128
S = 16384
D = 128
TD = D + 2                     # tile width: 128 feats + ones col + pad col
NCORES = 8
WIN = 64                       # segments per PSUM window
NWIN = S // WIN                # 256 global windows
NWIN_CORE = NWIN // NCORES     # 32 per core
GROUP = 32                     # tiles per DMA/logit batch
EPS = 1e-10

U8CAST = True                  # ship mask u8, SWDGE dma casts to bf16 on load
ACT_RED = ()                   # groups whose reduce runs on ACT (accum) — the
                               # hidden ACTIVATION_READ_ACCUMULATOR slice makes
                               # this a net loss (measured 445ns/tile)
GPS_MASK = ()                  # groups whose mask-mult runs on GPSIMD — big
                               # gpsimd streaming ops slow DVE ~15% via the
                               # shared SBUF port (measured v4); keep empty

LAST_EXEC_NS = None
LAST_RESULTS = None

_module_cache = {}


def _build_module(tpw):
    """tpw: tuple of NWIN_CORE tile counts (same for all cores)."""
    key = ("v8", GROUP, ACT_RED, GPS_MASK, tuple(tpw))
    if key in _module_cache:
        return _module_cache[key]

    import concourse.bass as bass  # noqa: F401
    import concourse.tile as tile
    from concourse import bacc, mybir
    from concourse.masks import make_identity

    f32 = mybir.dt.float32
    bf16 = mybir.dt.bfloat16
    AX = mybir.AxisListType
    ALU = mybir.AluOpType
    ACTF = mybir.ActivationFunctionType

    NT = int(sum(tpw))
    NG = (NT + GROUP - 1) // GROUP
    NTP = NG * GROUP

    # flat tile stream: (window, first, last) per real tile
    tiles = []
    for w, c in enumerate(tpw):
        for k in range(c):
            tiles.append((w, k == 0, k == c - 1))

    nc = bacc.Bacc(
        "TRN2",
        target_bir_lowering=False,
        debug=False,
        enable_asserts=True,
        num_devices=NCORES,
    )

    xp = nc.dram_tensor("xp", [NG * P, GROUP * TD], bf16, kind="ExternalInput")
    mdt = mybir.dt.uint8 if U8CAST else bf16
    maskg = nc.dram_tensor("maskg", [NG * P, GROUP * WIN], mdt, kind="ExternalInput")
    wall = nc.dram_tensor("wall", [P, NTP], f32, kind="ExternalInput")
    gwrep = nc.dram_tensor("gwrep", [P, GROUP * D], bf16, kind="ExternalInput")
    msgw = nc.dram_tensor("msgw", [D, D], f32, kind="ExternalInput")
    msgbrep = nc.dram_tensor("msgbrep", [P, D], f32, kind="ExternalInput")
    gatebrep = nc.dram_tensor("gatebrep", [P, 1], f32, kind="ExternalInput")
    prep = nc.dram_tensor("prep", [P, 1], f32, kind="ExternalInput")
    out = nc.dram_tensor("out", [NWIN_CORE * WIN, D], f32, kind="ExternalOutput")

    with tile.TileContext(nc) as tc:
        from contextlib import ExitStack

        with ExitStack() as ctx:
            const_pool = ctx.enter_context(tc.tile_pool(name="const", bufs=1))
            xs_pool = ctx.enter_context(tc.tile_pool(name="xs", bufs=4))
            mk_pool = ctx.enter_context(tc.tile_pool(name="mk", bufs=4))
            xw_pool = ctx.enter_context(tc.tile_pool(name="xw", bufs=2))
            tr_pool = ctx.enter_context(tc.tile_pool(name="tr", bufs=3))
            lg_pool = ctx.enter_context(tc.tile_pool(name="lg", bufs=6))
            g_pool = ctx.enter_context(tc.tile_pool(name="gm", bufs=3))
            ge_pool = ctx.enter_context(tc.tile_pool(name="ge", bufs=4))
            psA_pool = ctx.enter_context(tc.tile_pool(name="psA", bufs=4, space="PSUM"))
            ps2_pool = ctx.enter_context(tc.tile_pool(name="ps2", bufs=2, space="PSUM"))
            ph2_pool = ctx.enter_context(tc.tile_pool(name="ph2", bufs=3))

            gw_t = const_pool.tile([P, GROUP * D], bf16)
            nc.sync.dma_start(gw_t[:], gwrep[:, :])
            msgw_t = const_pool.tile([D, D], f32)
            nc.sync.dma_start(msgw_t[:], msgw[:, :])
            msgb_t = const_pool.tile([P, D], f32)
            nc.sync.dma_start(msgb_t[:], msgbrep[:, :])
            gateb_t = const_pool.tile([P, 1], f32)
            nc.sync.dma_start(gateb_t[:], gatebrep[:, :])
            p_t = const_pool.tile([P, 1], f32)
            nc.sync.dma_start(p_t[:], prep[:, :])
            ident = const_pool.tile([P, P], f32)
            make_identity(nc, ident[:])

            # hoisted: p*ln(w) for every tile in two ops
            w_t = const_pool.tile([P, NTP], f32)
            nc.sync.dma_start(w_t[:], wall[:, :])
            plw_t = const_pool.tile([P, NTP], f32)
            nc.scalar.activation(out=plw_t[:], in_=w_t[:], func=ACTF.Ln)
            nc.vector.tensor_scalar_mul(plw_t[:], plw_t[:], p_t[:, 0:1])

            gw3 = gw_t[:].rearrange("p (t d) -> p t d", d=D)

            chains = {}

            def emit_chain(g):
                gc = min(GROUP, NT - g * GROUP)
                xs = xs_pool.tile([P, gc * TD], bf16, tag="xs", name=f"xs{g}")
                nc.sync.dma_start(xs[:], xp[g * P : (g + 1) * P, 0 : gc * TD])
                xs3 = xs[:].rearrange("p (t d) -> p t d", d=TD)
                mk = mk_pool.tile([P, gc * WIN], bf16, tag="mk", name=f"mk{g}")
                if U8CAST:
                    nc.gpsimd.dma_start(mk[:], maskg[g * P : (g + 1) * P, 0 : gc * WIN])
                else:
                    nc.sync.dma_start(mk[:], maskg[g * P : (g + 1) * P, 0 : gc * WIN])
                xw = xw_pool.tile([P, gc * D], bf16, tag="xw", name=f"xw{g}")
                xw3 = xw[:].rearrange("p (t d) -> p t d", d=D)
                nc.vector.tensor_tensor(
                    out=xw3, in0=xs3[:, :, 0:D], in1=gw3[:, 0:gc, :], op=ALU.mult
                )
                logit = lg_pool.tile([P, gc], f32, tag="lg", name=f"lg{g}")
                t1 = tr_pool.tile([P, gc * 64], bf16, tag="t1", name=f"t1{g}")
                t13 = t1[:].rearrange("p (t d) -> p t d", d=64)
                nc.vector.tensor_tensor(
                    out=t13, in0=xw3[:, :, 0:64], in1=xw3[:, :, 64:128],
                    op=ALU.add,
                )
                t2 = tr_pool.tile([P, gc * 32], bf16, tag="t2", name=f"t2{g}")
                t23 = t2[:].rearrange("p (t d) -> p t d", d=32)
                nc.vector.tensor_tensor(
                    out=t23, in0=t13[:, :, 0:32], in1=t13[:, :, 32:64],
                    op=ALU.add,
                )
                t3 = tr_pool.tile([P, gc * 16], bf16, tag="t3", name=f"t3{g}")
                t33 = t3[:].rearrange("p (t d) -> p t d", d=16)
                nc.vector.tensor_tensor(
                    out=t33, in0=t23[:, :, 0:16], in1=t23[:, :, 16:32],
                    op=ALU.add,
                )
                nc.vector.reduce_sum(out=logit[:], in_=t33, axis=AX.X)
                logit2 = lg_pool.tile([P, gc], f32, tag="lg2", name=f"l2{g}")
                nc.vector.tensor_add(
                    logit2[:], logit[:], plw_t[:, g * GROUP : g * GROUP + gc]
                )
                gexb = ge_pool.tile([P, gc * WIN], bf16, tag="gexb", name=f"ge{g}")
                gexb3 = gexb[:].rearrange("p (t s) -> p t s", s=WIN)
                lg2b = logit2[:].unsqueeze(2).broadcast_to([P, gc, WIN])
                nc.scalar.activation(
                    out=gexb3, in_=lg2b, func=ACTF.Exp, bias=gateb_t[:, 0:1]
                )
                chains[g] = (xs3, mk, gexb, gc)

            def emit_gate(g):
                # deferred two groups behind emit_chain: by the time DVE reaches
                # this op in its in-order stream, ACT's exp(g) has had ample
                # time to land, so DVE doesn't stall on it
                xs3, mk, gexb, gc = chains[g]
                Gm = g_pool.tile([P, gc * WIN], bf16, tag="G", name=f"G{g}")
                eng = nc.gpsimd if g in GPS_MASK else nc.vector
                eng.tensor_tensor(out=Gm[:], in0=mk[:], in1=gexb[:], op=ALU.mult)
                chains[g] = (xs3, Gm)

            def emit_phase2(w, psA):
                sbA = ph2_pool.tile([WIN, D + 1], f32, tag="sbA", name=f"sbA{w}")
                nc.scalar.copy(sbA[:], psA[:])
                deno = ph2_pool.tile([WIN, 1], f32, tag="deno", name=f"dn{w}")
                nc.vector.tensor_scalar_add(deno[:], sbA[:, D : D + 1], EPS)
                rcp = ph2_pool.tile([WIN, 1], f32, tag="rcp", name=f"rc{w}")
                nc.vector.reciprocal(out=rcp[:], in_=deno[:])
                coef = ph2_pool.tile([WIN, 1], f32, tag="coef", name=f"cf{w}")
                nc.vector.tensor_tensor(
                    out=coef[:], in0=sbA[:, D : D + 1], in1=rcp[:], op=ALU.mult
                )
                psAT = ps2_pool.tile([P, WIN], f32, tag="AT", name=f"AT{w}")
                nc.tensor.transpose(
                    out=psAT[:], in_=sbA[:, 0:D], identity=ident[:WIN, :WIN]
                )
                sbAT = ph2_pool.tile([P, WIN], f32, tag="sbAT", name=f"sT{w}")
                nc.scalar.copy(sbAT[:], psAT[:])
                ps2 = ps2_pool.tile([WIN, D], f32, tag="out2", name=f"o2{w}")
                nc.tensor.matmul(
                    out=ps2[:], lhsT=sbAT[:], rhs=msgw_t[:], start=True, stop=True
                )
                outsb = ph2_pool.tile([WIN, D], f32, tag="outsb", name=f"ou{w}")
                nc.scalar.activation(
                    out=outsb[:], in_=ps2[:], func=ACTF.Copy, scale=rcp[:, 0:1]
                )
                bterm = ph2_pool.tile([WIN, D], f32, tag="bterm", name=f"bt{w}")
                nc.scalar.activation(
                    out=bterm[:], in_=msgb_t[:WIN, :], func=ACTF.Copy,
                    scale=coef[:, 0:1],
                )
                ofin = ph2_pool.tile([WIN, D], f32, tag="ofin", name=f"of{w}")
                nc.vector.tensor_add(ofin[:], outsb[:], bterm[:])
                nc.sync.dma_start(out[w * WIN : (w + 1) * WIN, :], ofin[:])

            cur = {}
            emit_chain(0)
            if NG > 1:
                emit_chain(1)
            for g in range(NG):
                if g + 2 < NG:
                    emit_chain(g + 2)
                emit_gate(g)
                xs3, Gm = chains.pop(g)
                for j in range(GROUP):
                    t = g * GROUP + j
                    if t >= NT:
                        break
                    w, first, last = tiles[t]
                    if first:
                        cur[w] = psA_pool.tile(
                            [WIN, D + 1], f32, tag="psA", name=f"psA{w}"
                        )
                    nc.tensor.matmul(
                        out=cur[w][:],
                        lhsT=Gm[:, j * WIN : (j + 1) * WIN],
                        rhs=xs3[:, j, 0 : D + 1],
                        start=first,
                        stop=last,
                    )
                    if last:
                        emit_phase2(w, cur.pop(w))

    nc.compile()
    _module_cache[key] = (nc, NT, NG)
    return _module_cache[key]


def _layout(x, idx, w):
    """Pad + reorder host arrays into the per-core device layouts.

    Returns (tpw, xdev, maskdev, wdev)."""
    n = idx.shape[0]
    bounds = np.searchsorted(idx, np.arange(0, S + 1, WIN)).astype(np.int64)
    counts = np.diff(bounds)                       # rows per global window [NWIN]
    cpw = counts.reshape(NCORES, NWIN_CORE)
    tpw = np.maximum(1, -(-cpw // P)).max(axis=0)  # tiles per window, shared
    NT = int(tpw.sum())
    NG = (NT + GROUP - 1) // GROUP
    NTP = NG * GROUP
    ROWS_CORE = NTP * P

    tile_off = np.zeros(NWIN_CORE + 1, dtype=np.int64)
    np.cumsum(tpw, out=tile_off[1:])

    wg = np.repeat(np.arange(NWIN, dtype=np.int64), counts)     # global window
    rank = np.arange(n, dtype=np.int64) - np.repeat(bounds[:-1], counts)
    core = wg // NWIN_CORE
    wl = wg % NWIN_CORE
    dest = core * ROWS_CORE + tile_off[wl] * P + rank

    xpad = np.zeros((NCORES * ROWS_CORE, TD), dtype=np.float32)
    xpad[:, D] = 1.0
    xpad[dest, 0:D] = x
    segl = (idx - wg * WIN).astype(np.int64)

    mask = np.zeros((NCORES * ROWS_CORE, WIN), dtype=np.uint8)
    mask[dest, segl] = 1
    wpad = np.ones(NCORES * ROWS_CORE, dtype=np.float32)
    wpad[dest] = w

    xdev = (
        xpad.astype(BF16)
        .reshape(NCORES, NG, GROUP, P, TD)
        .transpose(0, 1, 3, 2, 4)
        .reshape(NCORES, NG * P, GROUP * TD)
    )
    maskdev = (
        mask.reshape(NCORES, NG, GROUP, P, WIN)
        .transpose(0, 1, 3, 2, 4)
        .reshape(NCORES, NG * P, GROUP * WIN)
    )
    if not U8CAST:
        maskdev = maskdev.astype(BF16)
    wdev = np.ascontiguousarray(
        wpad.reshape(NCORES, NTP, P).transpose(0, 2, 1)
    )
    return tuple(int(t) for t in tpw), xdev, maskdev, wdev


def _ensure_ntff_hook():
    """The image's antenv package lacks axon_hooks; shim it so trace=True
    can register the ctypes NTFF hook from trn_agent_boot."""
    try:
        from antenv.axon_hooks import get_axon_ntff_profile_hook  # noqa: F401

        return True
    except ImportError:
        pass
    try:
        import types

        import antenv
        from trn_agent_boot.trn_boot import _ntff_profile_via_ctypes

        mod = types.ModuleType("antenv.axon_hooks")
        _hook = [None]
        mod.set_axon_ntff_profile_hook = lambda h: _hook.__setitem__(0, h)
        mod.get_axon_ntff_profile_hook = lambda: _hook[0]
        sys.modules["antenv.axon_hooks"] = mod
        antenv.axon_hooks = mod
        mod.set_axon_ntff_profile_hook(
            _ntff_profile_via_ctypes("/opt/axon/libaxon_pjrt.so")
        )
        return True
    except Exception as e:  # degrade to untraced run
        print(f"ntff hook install failed: {type(e).__name__}: {e}")
        return False


def kernel(x, index, weights, gate_w, gate_b, msg_w, msg_b, pow_p):
    global LAST_EXEC_NS, LAST_RESULTS

    x = np.ascontiguousarray(np.asarray(x, dtype=np.float32))
    idx = np.asarray(index).astype(np.int64).ravel()
    w = np.asarray(weights, dtype=np.float32).ravel()
    gate_w = np.asarray(gate_w, dtype=np.float32).reshape(D)
    gate_b = np.asarray(gate_b, dtype=np.float32).reshape(1)
    msg_w = np.ascontiguousarray(np.asarray(msg_w, dtype=np.float32))
    msg_b = np.asarray(msg_b, dtype=np.float32).reshape(D)
    pow_p = np.asarray(pow_p, dtype=np.float32).reshape(1)

    if not np.all(idx[1:] >= idx[:-1]):
        perm = np.argsort(idx, kind="stable")
        idx = idx[perm]
        x = x[perm]
        w = w[perm]

    tpw, xdev, maskdev, wdev = _layout(x, idx, w)

    gwrep = np.tile(gate_w[None, :], (P, GROUP)).astype(BF16)
    msgbrep = np.tile(msg_b[None, :], (P, 1)).astype(np.float32)
    gatebrep = np.full((P, 1), gate_b[0], dtype=np.float32)
    prep = np.full((P, 1), pow_p[0], dtype=np.float32)
    nc, NT, NG = _build_module(tpw)
    from concourse.bass_utils import run_bass_kernel_spmd

    in_maps = []
    for c in range(NCORES):
        in_maps.append(
            {
                "xp": np.ascontiguousarray(xdev[c]),
                "maskg": np.ascontiguousarray(maskdev[c]),
                "wall": wdev[c],
                "gwrep": gwrep,
                "msgw": msg_w,
                "msgbrep": msgbrep,
                "gatebrep": gatebrep,
                "prep": prep,
            }
        )

    trace = bool(os.environ.get("KERNEL_TRACE"))
    if trace:
        trace = _ensure_ntff_hook()
    res = run_bass_kernel_spmd(
        nc, in_maps, core_ids=list(range(NCORES)), trace=trace
    )
    LAST_RESULTS = res
    LAST_EXEC_NS = res.exec_time_ns

    out = np.concatenate([res.results[c]["out"] for c in range(NCORES)], axis=0)
    return out.astype(np.float32)


def kernel_numpy(x, index, weights, gate_w, gate_b, msg_w, msg_b, pow_p):
    """Host-side mirror of the device algorithm (debug only)."""
    x = np.asarray(x, dtype=np.float32)
    idx = np.asarray(index).astype(np.int64).ravel()
    w = np.asarray(weights, dtype=np.float32).ravel()
    xb = x.astype(BF16).astype(np.float32)
    gwb = np.asarray(gate_w, dtype=np.float32).astype(BF16).astype(np.float32)
    xw = (xb * gwb.reshape(1, D)).astype(BF16).astype(np.float32)
    t1 = (xw[:, 0:64] + xw[:, 64:128]).astype(BF16).astype(np.float32)
    t2 = (t1[:, 0:32] + t1[:, 32:64]).astype(BF16).astype(np.float32)
    gate = t2.sum(axis=1) + np.asarray(gate_b).reshape(1)[0]
    g = np.exp(gate + np.asarray(pow_p).reshape(1)[0] * np.log(w))
    g = g.astype(BF16).astype(np.float32)
    A = np.zeros((S, D), dtype=np.float64)
    den = np.zeros(S, dtype=np.float64)
    np.add.at(A, idx, g[:, None] * xb)
    np.add.at(den, idx, g)
    out = (A @ np.asarray(msg_w, dtype=np.float64)) / (den[:, None] + EPS)
    out = out + (den / (den + EPS))[:, None] * np.asarray(msg_b).reshape(1, D)
    return out.astype(np.float32)
